# revision 2
# baseline (speedup 1.0000x reference)
"""Trainium2 Bass kernel for the pooling+MLP model (nn_BaseModel_79250736546631).

Computation (per batch row b):
    mask  = (x[b, :200] > 0)
    avg   = mean(mask)                      # count/200, a per-row scalar
    user_vec = sum_h mask[h]*avg * emb[x[b,h]]
             = avg * (sum_h emb[x[b,h]] - n_zero * emb[0])
    h = concat(user_vec, emb[x[b,200]])
    out = sigmoid(relu(relu(h@W1+b1)@W2+b2)@W3+b3)

Sharding: data-parallel over batch across 8 NeuronCores; the 1M x 64
embedding table and the tiny MLP are replicated per core.

Single fused program per core (2048 rows, 16 tiles of 128):
  - Pool engine: per tile, 201 fire-and-forget indirect DMAs gather all
    embedding rows into SBUF [128, 201*64] (double buffered).
  - DVE: mask/count, strided reduce over the 200 history slots, H assembly,
    and the PSUM->SBUF copy of the transposed H.
  - PE: transpose of H + the 3 MLP matmuls (lhsT=W).
  - ACT: fused bias+ReLU / bias+sigmoid and the output DMA.
All tail work overlaps the (dominant) gather stream; one launch per pass.

Raw-bass hazards found on HW and handled here:
  - Dependent back-to-back DVE ops can read stale SBUF (no auto-sync in raw
    bass): the accum_out of the mask/count op and the tiny scalar chain
    (avg/n0/c/e0c) need intervening work before their consumers.
  - A DVE read of PSUM right after the PE transpose's sem fires can see
    stale PSUM at startup; a small delay op before the copy fixes it.
  - The ACT output DMA must wait on the sigmoid's completion sem.
"""

import sys

for _p in ("/opt/trn_rl_repo",):
    if _p not in sys.path:
        sys.path.insert(0, _p)

import numpy as np

P = 128
EMB = 64
HIST = 200
NIDX = HIST + 1
VOCAB = 1_000_000
B = 16384
NCORES = 8
B_CORE = B // NCORES  # 2048
TILES = B_CORE // P  # 16


def build_fused(vocab=VOCAB, b_core=B_CORE, reps=1):
    import concourse.bacc as bacc
    import concourse.bass as bass
    import concourse.mybir as mybir

    f32 = mybir.dt.float32
    i32 = mybir.dt.int32
    tiles = b_core // P
    total = tiles * reps
    NB = 2

    nc = bacc.Bacc(
        "TRN2", target_bir_lowering=False, debug=False,
        detect_race_conditions=False,
    )
    x_d = nc.dram_tensor("x", [b_core, NIDX], i32, kind="ExternalInput").ap()
    emb_d = nc.dram_tensor("emb", [vocab, EMB], f32, kind="ExternalInput").ap()
    w1_d = nc.dram_tensor("W1", [2 * EMB, 120], f32, kind="ExternalInput").ap()
    b1_d = nc.dram_tensor("b1", [120], f32, kind="ExternalInput").ap()
    w2_d = nc.dram_tensor("W2", [120, 60], f32, kind="ExternalInput").ap()
    b2_d = nc.dram_tensor("b2", [60], f32, kind="ExternalInput").ap()
    w3_d = nc.dram_tensor("W3", [60, 1], f32, kind="ExternalInput").ap()
    b3_d = nc.dram_tensor("b3", [1], f32, kind="ExternalInput").ap()
    e0b_d = nc.dram_tensor("emb0b", [P, EMB], f32, kind="ExternalInput").ap()
    idn_d = nc.dram_tensor("iden", [P, P], f32, kind="ExternalInput").ap()
    out_d = nc.dram_tensor("out", [tiles, P], f32, kind="ExternalOutput").ap()

    p0 = nc.alloc_psum_tensor("p0", [P, P], f32).ap()
    p1 = nc.alloc_psum_tensor("p1", [120, P], f32).ap()
    p2 = nc.alloc_psum_tensor("p2", [60, P], f32).ap()
    p3 = nc.alloc_psum_tensor("p3", [1, P], f32).ap()

    from contextlib import ExitStack

    with ExitStack() as ctx:
        x_t = ctx.enter_context(nc.sbuf_tensor("x_t", [P, NB * NIDX], i32))
        g_t = ctx.enter_context(nc.sbuf_tensor("g_t", [P, NB * NIDX * EMB], f32))
        s_t = ctx.enter_context(nc.sbuf_tensor("s_t", [P, NB * EMB], f32))
        xf_t = ctx.enter_context(nc.sbuf_tensor("xf_t", [P, HIST], f32))
        mask_t = ctx.enter_context(nc.sbuf_tensor("mask_t", [P, HIST], f32))
        cnt_t = ctx.enter_context(nc.sbuf_tensor("cnt_t", [P, 1], f32))
        avg_t = ctx.enter_context(nc.sbuf_tensor("avg_t", [P, 1], f32))
        n0_t = ctx.enter_context(nc.sbuf_tensor("n0_t", [P, 1], f32))
        c_t = ctx.enter_context(nc.sbuf_tensor("c_t", [P, 1], f32))
        e0c_t = ctx.enter_context(nc.sbuf_tensor("e0c_t", [P, EMB], f32))
        h_t = ctx.enter_context(nc.sbuf_tensor("h_t", [P, 2 * EMB], f32))
        ht_t = ctx.enter_context(nc.sbuf_tensor("ht_t", [P, P], f32))
        a1_t = ctx.enter_context(nc.sbuf_tensor("a1_t", [120, P], f32))
        a2_t = ctx.enter_context(nc.sbuf_tensor("a2_t", [60, P], f32))
        o_t = ctx.enter_context(nc.sbuf_tensor("o_t", [1, NB * P], f32))
        w1_t = ctx.enter_context(nc.sbuf_tensor("w1_t", [2 * EMB, 120], f32))
        w2_t = ctx.enter_context(nc.sbuf_tensor("w2_t", [120, 60], f32))
        w3_t = ctx.enter_context(nc.sbuf_tensor("w3_t", [60, 1], f32))
        b1_t = ctx.enter_context(nc.sbuf_tensor("b1_t", [120, 1], f32))
        b2_t = ctx.enter_context(nc.sbuf_tensor("b2_t", [60, 1], f32))
        b3_t = ctx.enter_context(nc.sbuf_tensor("b3_t", [1, 1], f32))
        e0b_t = ctx.enter_context(nc.sbuf_tensor("e0b_t", [P, EMB], f32))
        idn_t = ctx.enter_context(nc.sbuf_tensor("idn_t", [P, P], f32))
        xsem0 = ctx.enter_context(nc.semaphore("xsem0"))
        xsem1 = ctx.enter_context(nc.semaphore("xsem1"))
        gsem0 = ctx.enter_context(nc.semaphore("gsem0"))
        gsem1 = ctx.enter_context(nc.semaphore("gsem1"))
        vsem = ctx.enter_context(nc.semaphore("vsem"))
        hsem = ctx.enter_context(nc.semaphore("hsem"))
        dvc = ctx.enter_context(nc.semaphore("dvc"))
        pes = ctx.enter_context(nc.semaphore("pes"))
        acs = ctx.enter_context(nc.semaphore("acs"))
        odsem = ctx.enter_context(nc.semaphore("odsem"))
        wsem = ctx.enter_context(nc.semaphore("wsem"))
        dsem = ctx.enter_context(nc.semaphore("dsem"))
        block = ctx.enter_context(nc.Block())
        xsems = [xsem0, xsem1]
        gsems = [gsem0, gsem1]

        def gs(b, h):
            return slice((b * NIDX + h) * EMB, (b * NIDX + h + 1) * EMB)

        @block.sync
        def _(sy):
            sy.dma_start(out=w1_t[:], in_=w1_d[:]).then_inc(wsem, 16)
            sy.dma_start(out=w2_t[:], in_=w2_d[:]).then_inc(wsem, 16)
            sy.dma_start(out=w3_t[:], in_=w3_d[:]).then_inc(wsem, 16)
            sy.dma_start(out=b1_t[:], in_=b1_d[:, None]).then_inc(wsem, 16)
            sy.dma_start(out=b2_t[:], in_=b2_d[:, None]).then_inc(wsem, 16)
            sy.dma_start(out=b3_t[:], in_=b3_d[:, None]).then_inc(wsem, 16)
            sy.dma_start(out=e0b_t[:], in_=e0b_d[:]).then_inc(wsem, 16)
            sy.dma_start(out=idn_t[:], in_=idn_d[:]).then_inc(wsem, 16)
            for t in range(total):
                b = t % NB
                rows = slice((t % tiles) * P, (t % tiles + 1) * P)
                if t >= NB:
                    sy.wait_ge(vsem, t - NB + 1)
                sy.dma_start(
                    out=x_t[:, b * NIDX : (b + 1) * NIDX], in_=x_d[rows, :]
                ).then_inc(xsems[b], 16)

        @block.gpsimd
        def _(g):
            for t in range(total):
                b = t % NB
                g.wait_ge(xsems[b], 16 * (t // NB + 1))
                if t >= NB:
                    g.wait_ge(vsem, t - NB + 1)
                for h in range(NIDX):
                    inst = g.indirect_dma_start(
                        out=g_t[:, gs(b, h)],
                        out_offset=None,
                        in_=emb_d[:],
                        in_offset=bass.IndirectOffsetOnAxis(
                            ap=x_t[:, b * NIDX + h : b * NIDX + h + 1], axis=0
                        ),
                    )
                    # every gather fences via the buffer sem: DVE waits for the
                    # full per-tile count, no reliance on ring FIFO order
                    inst.then_inc(gsems[b], 16)

        @block.vector
        def _(v):
            v.wait_ge(wsem, 128)
            for t in range(total):
                b = t % NB
                # PSUM->SBUF copy of transposed H for tile t-1
                if t >= 1:
                    v.wait_ge(pes, 4 * (t - 1) + 1)
                    # drain delay: PE's sem fires at instruction retire; give
                    # the PSUM write time to land before DVE reads it
                    v.memset(e0c_t[:], 0.0)
                    v.tensor_copy(out=ht_t[:], in_=p0[:]).then_inc(dvc, 1)
                v.wait_ge(gsems[b], 16 * NIDX * (t // NB + 1))
                g_v = g_t[:, b * NIDX * EMB : (b + 1) * NIDX * EMB].rearrange(
                    "p (h j) -> p j h", j=EMB
                )
                # mask/cnt BEFORE the reduce: the 12800-cycle reduce separates
                # the accum_out write of cnt from its consumers (raw bass has
                # no auto-sync for the accumulator drain)
                v.tensor_copy(out=xf_t[:], in_=x_t[:, b * NIDX : b * NIDX + HIST])
                v.tensor_scalar(
                    out=mask_t[:],
                    in0=xf_t[:],
                    scalar1=0.0,
                    scalar2=None,
                    op0=mybir.AluOpType.is_gt,
                    op1=mybir.AluOpType.add,
                    accum_out=cnt_t[:],
                )
                v.reduce_sum(
                    out=s_t[:, b * EMB : (b + 1) * EMB],
                    in_=g_v[:, :, :HIST],
                    axis=mybir.AxisListType.X,
                )
                v.tensor_scalar_mul(out=avg_t[:], in0=cnt_t[:], scalar1=1.0 / HIST)
                v.tensor_scalar(
                    out=n0_t[:],
                    in0=cnt_t[:],
                    scalar1=float(HIST),
                    scalar2=-1.0,
                    op0=mybir.AluOpType.subtract,
                    op1=mybir.AluOpType.mult,
                )
                # spacers: adjacent dependent DVE ops read stale SBUF in raw
                # bass; big memsets let the producer's write retire first
                v.memset(mask_t[:, : HIST // 2], 0.0)
                v.memset(mask_t[:, HIST // 2 :], 0.0)
                v.tensor_tensor(
                    out=c_t[:], in0=avg_t[:], in1=n0_t[:], op=mybir.AluOpType.mult
                )
                v.tensor_scalar_mul(
                    out=h_t[:, :EMB],
                    in0=s_t[:, b * EMB : (b + 1) * EMB],
                    scalar1=avg_t[:, 0:1],
                )
                v.memset(xf_t[:, : HIST // 2], 0.0)
                v.tensor_scalar_mul(out=e0c_t[:], in0=e0b_t[:], scalar1=c_t[:, 0:1])
                v.memset(xf_t[:, HIST // 2 :], 0.0)
                v.tensor_tensor(
                    out=h_t[:, :EMB],
                    in0=h_t[:, :EMB],
                    in1=e0c_t[:],
                    op=mybir.AluOpType.subtract,
                )
                v.tensor_copy(
                    out=h_t[:, EMB:], in_=g_t[:, gs(b, HIST)]
                ).then_inc(hsem, 1)
                # separate instruction for the second sem update (walrus allows
                # only one sync update per compute instruction)
                v.memset(e0c_t[:, 0:1], 0.0).then_inc(vsem, 1)
            # final tile's PSUM copy
            t = total
            v.wait_ge(pes, 4 * (t - 1) + 1)
            v.memset(e0c_t[:], 0.0)
            v.tensor_copy(out=ht_t[:], in_=p0[:]).then_inc(dvc, 1)

        @block.tensor
        def _(pe):
            pe.wait_ge(wsem, 128)
            for t in range(total):
                pe.wait_ge(hsem, t + 1)
                if t >= 1:
                    pe.wait_ge(dvc, t)
                pe.transpose(out=p0[:], in_=h_t[:], identity=idn_t[:]).then_inc(
                    pes, 1
                )
                pe.wait_ge(dvc, t + 1)
                if t >= 1:
                    pe.wait_ge(acs, 3 * (t - 1) + 1)
                pe.matmul(
                    out=p1[:], lhsT=w1_t[:], rhs=ht_t[:], start=True, stop=True
                ).then_inc(pes, 1)
                pe.wait_ge(acs, 3 * t + 1)
                pe.matmul(
                    out=p2[:], lhsT=w2_t[:], rhs=a1_t[:], start=True, stop=True
                ).then_inc(pes, 1)
                pe.wait_ge(acs, 3 * t + 2)
                pe.matmul(
                    out=p3[:], lhsT=w3_t[:], rhs=a2_t[:], start=True, stop=True
                ).then_inc(pes, 1)

        @block.scalar
        def _(sc):
            sc.wait_ge(wsem, 128)
            for t in range(total):
                b = t % NB
                sc.wait_ge(pes, 4 * t + 2)
                sc.activation(
                    out=a1_t[:],
                    in_=p1[:],
                    func=mybir.ActivationFunctionType.Relu,
                    bias=b1_t[:, 0:1],
                ).then_inc(acs, 1)
                sc.wait_ge(pes, 4 * t + 3)
                sc.activation(
                    out=a2_t[:],
                    in_=p2[:],
                    func=mybir.ActivationFunctionType.Relu,
                    bias=b2_t[:, 0:1],
                ).then_inc(acs, 1)
                sc.wait_ge(pes, 4 * t + 4)
                if t >= NB:
                    sc.wait_ge(odsem, 16 * (t - NB + 1))
                sc.activation(
                    out=o_t[:, b * P : (b + 1) * P],
                    in_=p3[:],
                    func=mybir.ActivationFunctionType.Sigmoid,
                    bias=b3_t[:, 0:1],
                ).then_inc(acs, 1)
                # serialize the out-DMA behind the sigmoid's completion
                sc.wait_ge(acs, 3 * t + 3)
                sc.dma_start(
                    out=out_d[(t % tiles) : (t % tiles) + 1, :],
                    in_=o_t[:, b * P : (b + 1) * P],
                ).then_inc(odsem, 16)

    nc.compile()
    return nc


_NC_FUSED = None


def _get_nc():
    global _NC_FUSED
    if _NC_FUSED is None:
        _NC_FUSED = build_fused()
    return _NC_FUSED


def _in_maps(inputs):
    x32 = np.ascontiguousarray(np.asarray(inputs["x"], dtype=np.int32))
    emb = np.ascontiguousarray(np.asarray(inputs["emb"], dtype=np.float32))
    e0b = np.ascontiguousarray(np.broadcast_to(emb[0:1, :], (P, EMB)).copy())
    iden = np.eye(P, dtype=np.float32)
    stat = {
        "emb": emb,
        "emb0b": e0b,
        "iden": iden,
        "W1": np.ascontiguousarray(np.asarray(inputs["W1"], dtype=np.float32)),
        "b1": np.ascontiguousarray(np.asarray(inputs["b1"], dtype=np.float32)),
        "W2": np.ascontiguousarray(np.asarray(inputs["W2"], dtype=np.float32)),
        "b2": np.ascontiguousarray(np.asarray(inputs["b2"], dtype=np.float32)),
        "W3": np.ascontiguousarray(np.asarray(inputs["W3"], dtype=np.float32)),
        "b3": np.ascontiguousarray(np.asarray(inputs["b3"], dtype=np.float32)),
    }
    return [
        {"x": x32[c * B_CORE : (c + 1) * B_CORE], **stat} for c in range(NCORES)
    ]


def run(inputs, trace=False):
    """Fused single-program run on 8 cores. Returns (full [16384,1] f32, res)."""
    from concourse.bass_utils import run_bass_kernel_spmd

    nc = _get_nc()
    res = run_bass_kernel_spmd(
        nc, _in_maps(inputs), core_ids=list(range(NCORES)), trace=trace
    )
    outs = [
        np.asarray(res.results[c]["out"], dtype=np.float32).reshape(B_CORE, 1)
        for c in range(NCORES)
    ]
    return np.concatenate(outs, axis=0), res


def kernel(**inputs) -> np.ndarray:
    out, _ = run(inputs, trace=False)
    return out
